# revision 5
# baseline (speedup 1.0000x reference)
"""Bass/Tile TRN2 kernel for CrossAttention (B=2, N=4096, D=512, H=8, DH=64).

Sharding: batch*heads over 8 cores - core c handles batch c//4 and heads
(c%4)*2, (c%4)*2+1. Each core computes its two heads' attention and the
partial output projection O_h @ Wo_h; the host sums the 4 partials per batch.

v2 design (flipped PV):
- QK unchanged: per (slot, head) one matmul kt[64,128].T @ qt[64,512] ->
  st [128 keys, 512 queries] f32 PSUM (one bank per half-tile, bufs=5).
- exp: distributed across engines. Act path: exp activation straight to
  bf16 SBUF. Offload paths (Pool copy / DVE copy + DVE 4x tensor_scalar):
  Schraudolph fast-exp in bf16 via int16 affine bitcast.
- PV flipped: stationary = pt q-chunk [128 keys, 128 queries], moving =
  V||ones [128 keys, 65] -> acc [128 queries, 65] f32 PSUM accumulated
  over all 32 key chunks. 8 matmuls x 65 cols = 217ns/slot vs 427 before.
- Epilogue per i-block: per-partition reciprocal of denominators, one
  broadcast tensor_tensor normalize per head -> bf16, PE transposes to
  [dh, q] orientation, merged-head out-projection (one matmul per
  q-chunk) + bias via K=1 matmul, Pool copy to SBUF, DMA out.
"""

import sys

if "/opt/trn_rl_repo" not in sys.path:
    sys.path.insert(0, "/opt/trn_rl_repo")

import os
import numpy as np

B, N, D = 2, 4096, 512
H, DH = 8, 64
SCALE = DH ** -0.5
P = 128
IB = 512            # i block (queries)
NDC = D // P        # 4 contraction chunks for projections
NIB = N // IB       # 8
NJC = N // P        # 32 key chunks per i-block
NQ = IB // P        # 4 query chunks per i-block
NSLOT = NIB * NJC   # 256

SCH_C = 0.0434
SCH_A16 = float(2.0 ** 7 * np.log2(np.e) * SCALE)
SCH_B16 = float((127.0 - SCH_C) * 2.0 ** 7)

# exp-path assignment pattern over (2*slot+h) % len: A = Act-engine exp;
# P/D = Schraudolph offload (DVE copy to bf16, then int16-affine
# tensor_scalar on Pool for P / on DVE for D). GPSIMD cannot touch PSUM,
# so the PSUM-side copy is always DVE. i-block 0 is forced all-A: the DVE
# must run the projection staging copies there, and Act is idle-ish.
PAT = os.environ.get("K_PAT", "AADADADAADADADAD")  # 9 A + 7 D per 16
LAG_ACT = int(os.environ.get("K_LAG_ACT", "3"))
LAG_OFF = int(os.environ.get("K_LAG_OFF", "6"))

_CACHE: dict = {}


def _build():
    import concourse.mybir as mybir
    from concourse import bacc
    from concourse.bass import AP
    from concourse.tile import TileContext

    f32 = mybir.dt.float32
    f32r = mybir.dt.float32r
    bf16 = mybir.dt.bfloat16
    i16 = mybir.dt.int16
    Exp = mybir.ActivationFunctionType.Exp
    Mult = mybir.AluOpType.mult
    Add = mybir.AluOpType.add

    nc = bacc.Bacc("TRN2")
    xT = nc.dram_tensor("xT", [D, N], bf16, kind="ExternalInput")
    wq = nc.dram_tensor("wq", [D, 2 * DH], bf16, kind="ExternalInput")
    wk = nc.dram_tensor("wk", [D, 2 * DH], bf16, kind="ExternalInput")
    wv = nc.dram_tensor("wv", [D, 2 * DH], bf16, kind="ExternalInput")
    wo = nc.dram_tensor("wo", [2 * DH, D], f32r, kind="ExternalInput")
    bo = nc.dram_tensor("bo", [D], f32, kind="ExternalInput")
    ident = nc.dram_tensor("ident", [P, P], bf16, kind="ExternalInput")
    out = nc.dram_tensor("out", [N, D], f32, kind="ExternalOutput")
    K_DBG = os.environ.get("K_DBG", "0") == "1"
    if K_DBG:
        dbg_qt = nc.dram_tensor("dbg_qt", [P, IB], bf16, kind="ExternalOutput")
        dbg_kt = nc.dram_tensor("dbg_kt", [P, IB], bf16, kind="ExternalOutput")
        dbg_vt = nc.dram_tensor("dbg_vt", [P, NQ, 130], bf16,
                                kind="ExternalOutput")
        dbg_pt = nc.dram_tensor("dbg_pt", [P, IB], bf16, kind="ExternalOutput")
        dbg_on = nc.dram_tensor("dbg_on", [P, 2, NQ, DH], bf16,
                                kind="ExternalOutput")
        dbg_ot = nc.dram_tensor("dbg_ot", [P, NQ, P], f32r,
                                kind="ExternalOutput")
        dbg_acc = nc.dram_tensor("dbg_acc", [P, 2, NQ, 65], f32,
                                 kind="ExternalOutput")
        dbg_ri = nc.dram_tensor("dbg_ri", [P, 2, NQ], f32,
                                kind="ExternalOutput")

    with TileContext(nc) as tc, \
         tc.tile_pool(name="persist", bufs=1) as pp:
        xtb = [pp.tile([P, NDC, IB], bf16, name=f"xt{i}", tag=f"xt{i}")
               for i in range(NIB)]
        qtb = [pp.tile([P, IB], bf16, name=f"qt{i}", tag=f"qt{i}")
               for i in range(NIB)]
        ktb = [pp.tile([P, IB], bf16, name=f"kt{i}", tag=f"kt{i}")
               for i in range(NIB)]
        # V natural [keys, dh] + ones col per head: cols 0:64 v_h0, 64 ones,
        # 65:129 v_h1, 129 ones
        vtb = [pp.tile([P, NQ, 130], bf16, name=f"vt{i}", tag=f"vt{i}")
               for i in range(NIB)]
        wq_sb = pp.tile([P, NDC, 2 * DH], bf16, name="wq_sb", tag="wq")
        wk_sb = pp.tile([P, NDC, 2 * DH], bf16, name="wk_sb", tag="wk")
        wv_sb = pp.tile([P, NDC, 2 * DH], bf16, name="wv_sb", tag="wv")
        wo_sb = pp.tile([P, D], f32r, name="wo_sb", tag="wo")
        bo_sb = pp.tile([1, D], f32, name="bo_sb", tag="bos")
        bo_bc = pp.tile([P, D], f32, name="bo_bc", tag="bob")
        id_sb = pp.tile([P, P], bf16, name="id_sb", tag="id")

        # merged DMAs: one per tensor/block (HWDGE issue is ~625ns each, so
        # fewer, larger DMAs shorten the pipeline-fill critically)
        def w_ap(t):
            a = t[:, :]
            return AP(a.tensor, 0, [[2 * DH, P], [P * 2 * DH, NDC], [1, 2 * DH]])

        nc.sync.dma_start(wk_sb[:, :, :], w_ap(wk))
        xa = xT[:, :]
        # block 0 split per dc chunk so the first K-proj matmul can start
        # as soon as possible
        nc.sync.dma_start(xtb[0][:, 0, :], xT[0:P, 0:IB])
        nc.sync.dma_start(wq_sb[:, :, :], w_ap(wq))
        for dc in range(1, NDC):
            nc.sync.dma_start(xtb[0][:, dc, :],
                              xT[dc * P:(dc + 1) * P, 0:IB])
        nc.sync.dma_start(wv_sb[:, :, :], w_ap(wv))
        for b in range(1, NIB):
            nc.sync.dma_start(
                xtb[b][:, :, :],
                AP(xa.tensor, b * IB, [[N, P], [N * P, NDC], [1, IB]]))
        nc.sync.dma_start(wo_sb[:], wo[:, :])
        nc.sync.dma_start(bo_sb[:], bo[None, :])
        nc.sync.dma_start(id_sb[:], ident[:, :])
        nc.gpsimd.partition_broadcast(bo_bc[:], bo_sb[:])
        for b in range(NIB):
            nc.vector.memset(vtb[b][:, :, 64:65], 1.0)
            nc.vector.memset(vtb[b][:, :, 129:130], 1.0)

        with tc.tile_pool(name="ps", bufs=5, space="PSUM") as ps_pool, \
             tc.tile_pool(name="pacc", bufs=2, space="PSUM") as acc_pool, \
             tc.tile_pool(name="prt", bufs=1, space="PSUM") as rtp_pool, \
             tc.tile_pool(name="pt", bufs=16) as pt_pool, \
             tc.tile_pool(name="sg", bufs=5) as sg_pool, \
             tc.tile_pool(name="ep", bufs=2) as ep_pool, \
             tc.tile_pool(name="ob", bufs=4) as ob_pool:

            # ---------- helpers ----------
            def emit_k(b):
                pk = ps_pool.tile([P, IB], f32, tag="st", name="pk")
                for dc in range(NDC):
                    nc.tensor.matmul(pk[:], wk_sb[:, dc, :], xtb[b][:, dc, :],
                                     start=(dc == 0), stop=(dc == NDC - 1))
                nc.vector.tensor_copy(ktb[b][:], pk[:])

            def emit_q(b):
                pq = ps_pool.tile([P, IB], f32, tag="st", name="pq")
                for dc in range(NDC):
                    nc.tensor.matmul(pq[:], wq_sb[:, dc, :], xtb[b][:, dc, :],
                                     start=(dc == 0), stop=(dc == NDC - 1))
                nc.vector.tensor_copy(qtb[b][:], pq[:])

            def emit_v(b):
                pv = ps_pool.tile([P, NQ, P], f32, tag="st", name="pv")
                for kc in range(NQ):
                    for dc in range(NDC):
                        nc.tensor.matmul(
                            pv[:, kc, :],
                            xtb[b][:, dc, kc * P:(kc + 1) * P],
                            wv_sb[:, dc, :],
                            start=(dc == 0), stop=(dc == NDC - 1))
                for kc in range(NQ):
                    src = pv[:, kc, :]
                    dst = vtb[b][:, kc, :]
                    in_ap = AP(src.tensor, src.offset,
                               [src.ap[0], [DH, 2], [1, DH]])
                    out_ap = AP(dst.tensor, dst.offset,
                                [dst.ap[0], [DH + 1, 2], [1, DH]])
                    nc.vector.tensor_copy(out_ap, in_ap)

            # slot state
            pt_th = {}      # (t, h) -> pt tile
            acc_ib = {}     # ib -> (acc_h0, acc_h1)
            ep_state: dict = {}
            thunks = {}

            def at(slot, fn):
                thunks.setdefault(slot, []).append(fn)

            def emit_qk(t, h):
                ib, jc = divmod(t, NJC)
                b, kc = divmod(jc, NQ)
                st = ps_pool.tile([P, IB], f32, tag="st", name="st")
                nc.tensor.matmul(
                    st[:], ktb[b][DH * h:DH * (h + 1), kc * P:(kc + 1) * P],
                    qtb[ib][DH * h:DH * (h + 1), :],
                    start=True, stop=True, tile_position=(DH * h, 0))
                return st

            def emit_exp(t, h, st):
                path = "A" if t < NJC else PAT[(2 * t + h) % len(PAT)]
                pt = pt_pool.tile([P, IB], bf16, tag="pt", name="pt")
                if path == "A":
                    nc.scalar.activation(pt[:], st[:], Exp, scale=SCALE)
                elif path == "D":
                    # one-op Schraudolph: int16 affine straight off the PSUM
                    # scores (bf16 bit pattern = (127 + log2 p) * 128)
                    nc.vector.tensor_scalar(pt[:].bitcast(i16), st[:],
                                            SCH_A16, SCH_B16, Mult, Add)
                else:
                    stage = sg_pool.tile([P, IB], bf16, tag="sg", name="sg")
                    nc.vector.tensor_copy(stage[:], st[:])
                    nc.gpsimd.tensor_scalar(pt[:].bitcast(i16), stage[:],
                                            SCH_A16, SCH_B16, Mult, Add)
                pt_th[(t, h)] = pt
                if K_DBG and t == 0 and h == 0:
                    nc.sync.dma_start(dbg_pt[:, :], pt[:])
                return path

            def emit_pv(t, h):
                ib, jc = divmod(t, NJC)
                b = jc // NQ
                if (t, h) not in pt_th:
                    raise RuntimeError(f"missing pt {(t, h)}")
                pt = pt_th.pop((t, h))
                if jc == 0 and h == 0:
                    a0 = acc_pool.tile([P, NQ, 65], f32, tag="acc", name="a0")
                    a1 = acc_pool.tile([P, NQ, 65], f32, tag="acc", name="a1")
                    acc_ib[ib] = (a0, a1)
                acc = acc_ib[ib][h]
                vt = vtb[b][:, jc % NQ, 65 * h:65 * h + 65]
                for q in range(NQ):
                    # exactly ONE start=True per acc tile: start marks the
                    # whole 2KB PSUM bank pending-zero, so a start on q>0
                    # would wipe sibling chunks' already-accumulated data.
                    # The q>0 jc==0 writes consume q0's bank-wide pending
                    # flags and land as first-writes.
                    nc.tensor.matmul(acc[:, q, :], pt[:, q * P:(q + 1) * P],
                                     vt, start=(jc == 0 and q == 0),
                                     stop=(jc == NJC - 1 and q == NQ - 1),
                                     skip_group_check=True)

            def sched_epilogue(ib, t0):
                """Epilogue of i-block ib, spread over slots t0+4.."""
                def recips():
                    a0, a1 = acc_ib[ib]
                    rinv = ep_pool.tile([P, 2, NQ], f32, tag="ri", name="ri")
                    nc.vector.reciprocal(rinv[:, 0, :], a0[:, :, 64])
                    nc.vector.reciprocal(rinv[:, 1, :], a1[:, :, 64])
                    if K_DBG and ib == 0:
                        asb = ep_pool.tile([P, 2, NQ, 65], f32, tag="dbga",
                                           name="dbga")
                        nc.vector.tensor_copy(asb[:, 0], a0[:])
                        nc.vector.tensor_copy(asb[:, 1], a1[:])
                        nc.sync.dma_start(dbg_acc[:, :, :, :], asb[:])
                        nc.sync.dma_start(dbg_ri[:, :, :], rinv[:])
                    ep_state["rinv"] = rinv

                def norm():
                    a0, a1 = acc_ib.pop(ib)
                    rinv = ep_state.pop("rinv")
                    onrm = ep_pool.tile([P, 2, NQ, DH], bf16, tag="on",
                                        name="on")
                    for h, a in ((0, a0), (1, a1)):
                        rv = rinv[:, h, :]
                        rb = AP(rv.tensor, rv.offset,
                                [rv.ap[0], [1, NQ], [0, DH]])
                        nc.vector.tensor_tensor(onrm[:, h, :, :],
                                                a[:, :, 0:DH], rb, Mult)
                    if K_DBG and ib == 0:
                        nc.sync.dma_start(dbg_on[:, :, :, :], onrm[:])
                    ep_state["onrm"] = onrm

                def trans():
                    onrm = ep_state.pop("onrm")
                    rtp = rtp_pool.tile([P, NQ, P], bf16, tag="ep",
                                        name="rtp")
                    for h in range(2):
                        for q in range(NQ):
                            # (walrus: transpose stationary AP must have a
                            # single free dim, so one per head)
                            nc.tensor.transpose(
                                rtp[DH * h:DH * (h + 1), q, :],
                                onrm[:, h, q, :], id_sb[:])
                    # f32r so the out-proj matmul's operands dtype-match
                    # (walrus: f32/f32r may not mix with other dtypes)
                    ots = ep_pool.tile([P, NQ, P], f32r, tag="ots", name="ots")
                    nc.vector.tensor_copy(ots[:], rtp[:])
                    if K_DBG and ib == 0:
                        nc.sync.dma_start(dbg_ot[:, :, :], ots[:])
                    ep_state["ots"] = ots

                def outq(q, last):
                    ots = ep_state["ots"] if not last else ep_state.pop("ots")
                    # last i-block: QK traffic is done, so borrow the idle st
                    # bufs to pipeline the 4 out-proj chunks instead of
                    # serializing on the single shared "ep" bank
                    pool, tag = ((ps_pool, "st") if ib == NIB - 1 else
                                 (rtp_pool, "ep"))
                    ppx = pool.tile([P, D], f32, tag=tag, name="ppx")
                    nc.tensor.matmul(ppx[:], ots[:, q, :], wo_sb[:],
                                     start=True, stop=True)
                    osb = ob_pool.tile([P, D], f32, tag="ob", name="osb")
                    # bias folded into the PSUM->SBUF move (same DVE cost as
                    # a plain copy; saves a PE matmul per chunk)
                    nc.vector.tensor_tensor(osb[:], ppx[:], bo_bc[:], Add)
                    nc.sync.dma_start(
                        out[ib * IB + q * P:ib * IB + (q + 1) * P, :], osb[:])

                tight = ib == NIB - 1
                d = 3 if tight else 4
                at(t0 + d, recips)
                at(t0 + d + 1, norm)
                at(t0 + d + 2, trans)
                for q in range(NQ):
                    s = t0 + d + 3 + q if tight else t0 + 8 + 2 * q
                    at(s, lambda q=q, last=(q == NQ - 1): outq(q, last))

            # ---------- schedule ----------
            emit_k(0)
            emit_q(0)
            emit_v(0)
            for b in range(1, NIB):
                at(max(0, 4 * b - 6), lambda b=b: emit_k(b))
                at(max(1, 4 * b - 5), lambda b=b: emit_v(b))
            for ib in range(1, NIB):
                at((ib - 1) * NJC + 16, lambda b=ib: emit_q(b))
            for ib in range(NIB):
                sched_epilogue(ib, (ib + 1) * NJC)

            # pending PV halves: list of (t, h, release_slot)
            pend = []

            def release_slot(t, h, path):
                m, jc = divmod(t, NJC)
                lag = LAG_ACT if path == "A" else LAG_OFF
                # all of block m's PVs MUST be emitted before its epilogue
                # recips thunk — thunks are emitted first within a slot, so
                # a PV landing on that slot is read-before-write and its
                # chunk is silently dropped.
                clamp = 2 if m == NIB - 1 else 3
                r = min(t + lag, (m + 1) * NJC + clamp)
                if m >= 1 and jc < NQ:
                    # hold first-4-slot PVs until prior block's acc is freed
                    r = max(r, m * NJC + 6)
                return r

            TAIL = 24
            for u in range(NSLOT + TAIL):
                for fn in thunks.pop(u, ()):
                    fn()
                if u < NSLOT:
                    for h in range(2):
                        st = emit_qk(u, h)
                        path = emit_exp(u, h, st)
                        pend.append((u, h, release_slot(u, h, path)))
                ready = [e for e in pend if e[2] <= u]
                if u >= NSLOT:
                    # drain everything in order
                    ready = sorted(pend, key=lambda e: (e[0], e[1]))[:8]
                for e in ready:
                    pend.remove(e)
                    emit_pv(e[0], e[1])
            assert not pend, f"undrained PV: {pend[:4]}"
            for s in sorted(thunks):
                for fn in thunks[s]:
                    fn()
            thunks.clear()
            if K_DBG:
                nc.sync.dma_start(dbg_qt[:, :], qtb[0][:])
                nc.sync.dma_start(dbg_kt[:, :], ktb[0][:])
                nc.sync.dma_start(dbg_vt[:, :, :], vtb[0][:])

    nc.compile()
    return nc


def _get_nc():
    if "nc" not in _CACHE:
        _CACHE["nc"] = _build()
    return _CACHE["nc"]


def kernel(x, Wq, Wk, Wv, Wo, bo):
    import ml_dtypes
    from concourse.bass_utils import run_bass_kernel_spmd

    bf = ml_dtypes.bfloat16

    x = np.asarray(x, dtype=np.float32)
    Wq = np.asarray(Wq, dtype=np.float32)
    Wk = np.asarray(Wk, dtype=np.float32)
    Wv = np.asarray(Wv, dtype=np.float32)
    Wo = np.asarray(Wo, dtype=np.float32)
    bo = np.asarray(bo, dtype=np.float32)

    nc = _get_nc()

    xTs = [np.ascontiguousarray(x[b].T).astype(bf) for b in range(B)]
    eye = np.eye(P, dtype=bf)
    zeros_bo = np.zeros_like(bo)
    in_maps = []
    for c in range(8):
        b, p = c // 4, c % 4
        sl = slice(p * 2 * DH, (p + 1) * 2 * DH)
        in_maps.append({
            "xT": xTs[b],
            "wq": np.ascontiguousarray(Wq[:, sl]).astype(bf),
            "wk": np.ascontiguousarray(Wk[:, sl]).astype(bf),
            "wv": np.ascontiguousarray(Wv[:, sl]).astype(bf),
            "wo": np.ascontiguousarray(Wo[sl, :]),
            "bo": bo if p == 0 else zeros_bo,
            "ident": eye,
        })

    try:
        res = run_bass_kernel_spmd(nc, in_maps, core_ids=list(range(8)))
    except Exception:
        # transient device wedge - retry once
        import time as _time
        _time.sleep(45)
        res = run_bass_kernel_spmd(nc, in_maps, core_ids=list(range(8)))
    parts = [res.results[c]["out"] for c in range(8)]
    full = np.stack([
        parts[0] + parts[1] + parts[2] + parts[3],
        parts[4] + parts[5] + parts[6] + parts[7],
    ]).astype(np.float32)
    return full
